# revision 7
# baseline (speedup 1.0000x reference)
"""Trainium2 Bass kernel for a 6-layer dense transformer encoder.

Sharding: data-parallel over batch — B=8 sequences, one per NeuronCore.
Each core runs the full model on its [1024] token slice; weights are
replicated. No collectives; host gathers the per-core [1,5] outputs.

Device dataflow per core (all matmuls bf16 with fp32 PSUM accumulation):
  - embedding gather (indirect DMA) + pos add + LN0
  - per layer: PE-transpose z -> xT; QT/KT/V projections; per head
    scoresT = K @ Q^T (k on partitions), exp on ScalarE, ctx via
    lhsT=expT stationary with a ones-column appended to V to produce
    softmax denominators for free; normalize; LN.
  - mean-pool (free-dim reduce on xT) -> MLP -> [1, 5]

LayerNorm gain/bias and the 1/sqrt(d) score scale and 1/S pooling are
folded into the weights on the host (exact, fp64), so the device LN is
just (x - mean) * rsqrt(var + eps).
"""

import numpy as np
import ml_dtypes

import concourse.bass as bass
import concourse.tile as tile
import concourse.mybir as mybir
from concourse import bacc
from concourse.bass_utils import run_bass_kernel_spmd
from concourse.masks import make_identity

V, E, H, L = 32000, 768, 12, 6
HID, OUT = 3072, 5
B, S = 8, 1024
D = 64
EPS = 1e-5
P = 128
KO = E // P    # 6 contraction tiles over the model dim
ST = S // P    # 8 sequence tiles of 128
NQ = S // 512  # 2 free-dim halves of the sequence
NH = HID // 512
KH = HID // P  # 24

f32 = mybir.dt.float32
bf16 = mybir.dt.bfloat16
i32 = mybir.dt.int32
AF = mybir.ActivationFunctionType
ALU = mybir.AluOpType

_NC_CACHE = {}


def _layernorm(nc, sb_small, in_ap, out_ap, eps_tile):
    """out = (in - mean(in)) * rsqrt(var(in) + eps) along 768-wide free dim."""
    stats = sb_small.tile([P, 3, 6], f32, tag="ln_stats")
    xv = in_ap.rearrange("p (c d) -> p c d", c=3)
    for c in range(3):
        nc.vector.bn_stats(out=stats[:, c, :], in_=xv[:, c, :])
    mv = sb_small.tile([P, 2], f32, tag="ln_mv")
    nc.vector.bn_aggr(out=mv[:], in_=stats[:])
    rstd = sb_small.tile([P, 1], f32, tag="ln_rstd")
    nc.scalar.activation(out=rstd[:], in_=mv[:, 1:2], func=AF.Sqrt,
                         bias=eps_tile[:], scale=1.0)
    nc.vector.reciprocal(out=rstd[:], in_=rstd[:])
    nc.vector.tensor_scalar(out_ap, in_ap, mv[:, 0:1], rstd[:],
                            ALU.subtract, ALU.mult)


def build_nc(use_bq, use_bk, use_bv, n_layers=L, with_head=True, with_attn=True):
    nc = bacc.Bacc("TRN2", target_bir_lowering=False, debug=False)

    idx_d = nc.dram_tensor("idx", [S, 1], i32, kind="ExternalInput")
    tok_d = nc.dram_tensor("tok", [V, E], f32, kind="ExternalInput")
    pos_d = nc.dram_tensor("pos", [S, E], f32, kind="ExternalInput")
    wq_d = nc.dram_tensor("wq", [L, E, E], bf16, kind="ExternalInput")
    wk_d = nc.dram_tensor("wk", [L, E, E], bf16, kind="ExternalInput")
    wv_d = nc.dram_tensor("wv", [L, E, E], bf16, kind="ExternalInput")
    bq_d = nc.dram_tensor("bq", [L, E], f32, kind="ExternalInput")
    bk_d = nc.dram_tensor("bk", [L, E], f32, kind="ExternalInput")
    bv_d = nc.dram_tensor("bv", [L, E], f32, kind="ExternalInput")
    w1_d = nc.dram_tensor("w1", [E, HID], bf16, kind="ExternalInput")
    b1_d = nc.dram_tensor("b1", [1, HID], f32, kind="ExternalInput")
    w2_d = nc.dram_tensor("w2", [HID, OUT], bf16, kind="ExternalInput")
    b2_d = nc.dram_tensor("b2", [1, OUT], f32, kind="ExternalInput")
    out_d = nc.dram_tensor("out", [1, OUT], f32, kind="ExternalOutput")

    from contextlib import ExitStack
    with tile.TileContext(nc) as tc:
        with ExitStack() as ctx:
            ent = ctx.enter_context
            consts = ent(tc.tile_pool(name="consts", bufs=1))
            sb_small = ent(tc.tile_pool(name="sb_small", bufs=4))
            embp = ent(tc.tile_pool(name="embp", bufs=2))
            zp = ent(tc.tile_pool(name="zp", bufs=2))
            xnewp = ent(tc.tile_pool(name="xnewp", bufs=1))
            xtp = ent(tc.tile_pool(name="xtp", bufs=1))
            qtp = ent(tc.tile_pool(name="qtp", bufs=1))
            ktp = ent(tc.tile_pool(name="ktp", bufs=1))
            vp = ent(tc.tile_pool(name="vp", bufs=2))
            expp = ent(tc.tile_pool(name="expp", bufs=2))
            wp = ent(tc.tile_pool(name="wp", bufs=1))
            headp = ent(tc.tile_pool(name="headp", bufs=1))
            ps_proj = ent(tc.tile_pool(name="ps_proj", bufs=2, space="PSUM"))
            ps_sc = ent(tc.tile_pool(name="ps_sc", bufs=2, space="PSUM"))
            ps_sm = ent(tc.tile_pool(name="ps_sm", bufs=2, space="PSUM"))

            ident = consts.tile([P, P], bf16)
            make_identity(nc, ident[:])
            eps_t = consts.tile([P, 1], f32)
            nc.vector.memset(eps_t[:], EPS)
            idx_sb = consts.tile([P, ST], i32)
            nc.sync.dma_start(idx_sb[:], idx_d.ap().rearrange("(t p) o -> p (t o)", p=P))

            # ---- embedding + LN0 -> z ----
            z = zp.tile([P, ST, E], bf16, tag="z")
            for st in range(ST):
                emb = embp.tile([P, E], f32, tag="emb")
                nc.gpsimd.indirect_dma_start(
                    out=emb[:], out_offset=None, in_=tok_d.ap(),
                    in_offset=bass.IndirectOffsetOnAxis(ap=idx_sb[:, st:st + 1], axis=0),
                )
                pos = embp.tile([P, E], f32, tag="pos")
                nc.sync.dma_start(pos[:], pos_d.ap()[st * P:(st + 1) * P, :])
                nc.vector.tensor_add(out=emb[:], in0=emb[:], in1=pos[:])
                _layernorm(nc, sb_small, emb[:], z[:, st, :], eps_t)

            # ---- transformer layers ----
            for l in range(n_layers):
                wq_sb = wp.tile([P, KO, E], bf16, tag="wq")
                nc.sync.dma_start(wq_sb[:], wq_d.ap()[l].rearrange("(ko p) f -> p ko f", p=P))
                wk_sb = wp.tile([P, KO, E], bf16, tag="wk")
                nc.sync.dma_start(wk_sb[:], wk_d.ap()[l].rearrange("(ko p) f -> p ko f", p=P))
                wv_sb = wp.tile([P, KO, E], bf16, tag="wv")
                nc.sync.dma_start(wv_sb[:], wv_d.ap()[l].rearrange("(ko p) f -> p ko f", p=P))
                if use_bq:
                    bq_sb = sb_small.tile([P, KO], f32, tag="bq")
                    nc.sync.dma_start(bq_sb[:], bq_d.ap()[l].rearrange("(ko p) -> p ko", p=P))
                if use_bk:
                    bk_sb = sb_small.tile([P, KO], f32, tag="bk")
                    nc.sync.dma_start(bk_sb[:], bk_d.ap()[l].rearrange("(ko p) -> p ko", p=P))
                if use_bv:
                    bv_bc = sb_small.tile([P, E], f32, tag="bv")
                    src = bv_d.ap()[l]
                    nc.sync.dma_start(bv_bc[:], bass.AP(
                        tensor=src.tensor, offset=src.offset, ap=[[0, P], *src.ap]))

                # z -> xT (PE transpose per 128x128 block)
                xT = xtp.tile([P, KO, S], bf16, tag="xT")
                for ko in range(KO):
                    for st in range(ST):
                        tp = ps_sm.tile([P, P], bf16, tag="sm")
                        nc.tensor.transpose(tp[:], z[:, st, ko * P:(ko + 1) * P], ident[:])
                        nc.vector.tensor_copy(xT[:, ko, st * P:(st + 1) * P], tp[:])

                # QT/KT projections: [e_out partitions, s free]
                QT = qtp.tile([P, KO, S], bf16, tag="QT")
                KT = ktp.tile([P, KO, S], bf16, tag="KT")
                for dst, w_sb, b_sb in ((QT, wq_sb, bq_sb if use_bq else None),
                                        (KT, wk_sb, bk_sb if use_bk else None)):
                    for eo in range(KO):
                        for qh in range(NQ):
                            pq = ps_proj.tile([P, 512], f32, tag="proj")
                            for ko in range(KO):
                                nc.tensor.matmul(
                                    pq[:], w_sb[:, ko, eo * P:(eo + 1) * P],
                                    xT[:, ko, qh * 512:(qh + 1) * 512],
                                    start=(ko == 0), stop=(ko == KO - 1))
                            o = dst[:, eo, qh * 512:(qh + 1) * 512]
                            if b_sb is not None:
                                nc.vector.tensor_scalar_add(o, pq[:], b_sb[:, eo:eo + 1])
                            else:
                                nc.vector.tensor_copy(o, pq[:])

                # V projection in natural layout with ones column: [s part, kt, h, 65]
                Vp = vp.tile([P, ST, H, D + 1], bf16, tag="Vp")
                nc.vector.memset(Vp[:, :, :, D:D + 1], 1.0)
                for st in range(ST):
                    for half in range(2):
                        pv = ps_proj.tile([P, 384], f32, tag="proj")
                        for ko in range(KO):
                            nc.tensor.matmul(
                                pv[:], xT[:, ko, st * P:(st + 1) * P],
                                wv_sb[:, ko, half * 384:(half + 1) * 384],
                                start=(ko == 0), stop=(ko == KO - 1))
                        o = Vp[:, st, half * 6:(half + 1) * 6, 0:D]
                        pvv = pv[:].rearrange("p (h d) -> p h d", h=6)
                        if use_bv:
                            bvv = bv_bc[:, half * 384:(half + 1) * 384].rearrange(
                                "p (h d) -> p h d", h=6)
                            nc.vector.tensor_tensor(out=o, in0=pvv, in1=bvv, op=ALU.add)
                        else:
                            nc.vector.tensor_copy(o, pvv)

                # attention + ctx + normalize
                xnew = xnewp.tile([P, ST, E], bf16, tag="xnew")
                for h in range(H):
                    po = (h % 2) * D
                    eo = h // 2
                    expT = expp.tile([P, ST, S], bf16, tag="expT")
                    for kt in range(ST):
                        sc = ps_sc.tile([P, 1024], f32, tag="sc")
                        for qh in range(NQ):
                            nc.tensor.matmul(
                                sc[:, qh * 512:(qh + 1) * 512],
                                KT[po:po + D, eo, kt * P:(kt + 1) * P],
                                QT[po:po + D, eo, qh * 512:(qh + 1) * 512])
                        nc.scalar.activation(out=expT[:, kt, :], in_=sc[:],
                                             func=AF.Exp, bias=0.0, scale=1.0)
                    for qt in range(ST):
                        ct = ps_sm.tile([P, D + 1], f32, tag="sm")
                        for kt in range(ST):
                            nc.tensor.matmul(
                                ct[:], expT[:, kt, qt * P:(qt + 1) * P],
                                Vp[:, kt, h, :],
                                start=(kt == 0), stop=(kt == ST - 1))
                        rec = sb_small.tile([P, 1], f32, tag="rec")
                        nc.vector.reciprocal(rec[:], ct[:, D:D + 1])
                        nc.vector.tensor_scalar_mul(
                            xnew[:, qt, h * D:(h + 1) * D], ct[:, 0:D], rec[:])

                # LN -> z for next layer
                z = zp.tile([P, ST, E], bf16, tag="z")
                for st in range(ST):
                    _layernorm(nc, sb_small, xnew[:, st, :], z[:, st, :], eps_t)

            # ---- head: mean-pool (sum; 1/1024 folded in w1) + MLP ----
            if not with_head:
                o_sb = headp.tile([1, OUT], f32, tag='o_sb')
                nc.vector.memset(o_sb[:], 0.0)
                nc.vector.tensor_scalar_add(o_sb[0, 0:1], z[0, 0, 0:1], 0.0)
                nc.sync.dma_start(out_d.ap(), o_sb[:])
            xTf = xtp.tile([P, KO, S], bf16, tag="xT")
            for ko in range(KO):
                for st in range(ST):
                    tp = ps_sm.tile([P, P], bf16, tag="sm")
                    nc.tensor.transpose(tp[:], z[:, st, ko * P:(ko + 1) * P], ident[:])
                    nc.vector.tensor_copy(xTf[:, ko, st * P:(st + 1) * P], tp[:])
            pooled_f = headp.tile([P, KO], f32, tag="pooled_f")
            nc.vector.reduce_sum(out=pooled_f[:], in_=xTf[:], axis=mybir.AxisListType.X)
            pooled = headp.tile([P, KO], bf16, tag="pooled")
            nc.vector.tensor_copy(pooled[:], pooled_f[:])

            hT_pre = headp.tile([P, KH], f32, tag="hT_pre")
            for nt in range(NH):
                w1_sb = wp.tile([P, KO, 512], bf16, tag="wq")
                nc.sync.dma_start(
                    w1_sb[:], w1_d.ap().rearrange("(ko p) f -> p ko f", p=P)[:, :, nt * 512:(nt + 1) * 512])
                # hT block = W1_tile.T @ pooledT  -> lands already transposed
                for hsub in range(4):
                    phT = ps_sm.tile([P, 1], f32, tag="sm")
                    for ko in range(KO):
                        nc.tensor.matmul(phT[:], w1_sb[:, ko, hsub * P:(hsub + 1) * P],
                                         pooled[:, ko:ko + 1],
                                         start=(ko == 0), stop=(ko == KO - 1))
                    nc.vector.tensor_copy(hT_pre[:, nt * 4 + hsub:nt * 4 + hsub + 1], phT[:])
            b1T = headp.tile([P, KH], f32, tag="b1T")
            nc.sync.dma_start(b1T[:], b1_d.ap()[0].rearrange("(ko p) -> p ko", p=P))
            nc.vector.tensor_add(out=hT_pre[:], in0=hT_pre[:], in1=b1T[:])
            hT = headp.tile([P, KH], bf16, tag="hT")
            nc.vector.tensor_scalar_max(hT[:], hT_pre[:], 0.0)

            w2_sb = headp.tile([P, KH, OUT], bf16, tag="w2_sb")
            nc.sync.dma_start(w2_sb[:], w2_d.ap().rearrange("(ko p) f -> p ko f", p=P))
            b2_sb = headp.tile([1, OUT], f32, tag="b2_sb")
            nc.sync.dma_start(b2_sb[:], b2_d.ap())
            po2 = ps_sm.tile([1, OUT], f32, tag="sm")
            for ko in range(KH):
                nc.tensor.matmul(po2[:], hT[:, ko:ko + 1], w2_sb[:, ko, :],
                                 start=(ko == 0), stop=(ko == KH - 1))
            o_sb = headp.tile([1, OUT], f32, tag="o_sb")
            nc.vector.tensor_add(out=o_sb[:], in0=po2[:], in1=b2_sb[:])
            nc.sync.dma_start(out_d.ap(), o_sb[:])

    nc.compile()
    return nc


def _get_nc(use_bq, use_bk, use_bv):
    key = (use_bq, use_bk, use_bv)
    if key not in _NC_CACHE:
        _NC_CACHE[key] = build_nc(*key)
    return _NC_CACHE[key]


def prep_weights(inputs):
    """Fold LN affine params, score scale and pooling mean into the weights."""
    f8 = np.float64
    Wq = np.asarray(inputs["Wq"], f8)
    Wk = np.asarray(inputs["Wk"], f8)
    Wv = np.asarray(inputs["Wv"], f8)
    bq = np.asarray(inputs["bq"], f8)
    bk = np.asarray(inputs["bk"], f8)
    bv = np.asarray(inputs["bv"], f8)
    lng = np.asarray(inputs["lng"], f8)
    lnb = np.asarray(inputs["lnb"], f8)
    g_prev = np.concatenate([np.asarray(inputs["ln0_g"], f8)[None], lng[:L - 1]], 0)
    b_prev = np.concatenate([np.asarray(inputs["ln0_b"], f8)[None], lnb[:L - 1]], 0)

    scale = 1.0 / np.sqrt(D)
    wq_eff = g_prev[:, :, None] * Wq * scale
    bq_eff = (bq + np.einsum("le,lef->lf", b_prev, Wq)) * scale
    wk_eff = g_prev[:, :, None] * Wk
    bk_eff = bk + np.einsum("le,lef->lf", b_prev, Wk)
    wv_eff = g_prev[:, :, None] * Wv
    bv_eff = bv + np.einsum("le,lef->lf", b_prev, Wv)

    W1 = np.asarray(inputs["W1"], f8)
    w1_eff = lng[L - 1][:, None] * W1 / S
    b1_eff = np.asarray(inputs["b1"], f8) + lnb[L - 1] @ W1

    bf = ml_dtypes.bfloat16
    return {
        "wq": wq_eff.astype(bf), "wk": wk_eff.astype(bf), "wv": wv_eff.astype(bf),
        "bq": bq_eff.astype(np.float32), "bk": bk_eff.astype(np.float32),
        "bv": bv_eff.astype(np.float32),
        "w1": w1_eff.astype(bf), "b1": b1_eff.astype(np.float32)[None, :],
        "w2": np.asarray(inputs["W2"], f8).astype(bf),
        "b2": np.asarray(inputs["b2"], f8).astype(np.float32)[None, :],
        "tok": np.asarray(inputs["tok_emb"], np.float32),
        "pos": np.asarray(inputs["pos_emb"], np.float32)[:S],
    }


def kernel(**inputs) -> np.ndarray:
    w = prep_weights(inputs)
    use_bq = bool(np.any(w["bq"]))
    use_bk = bool(np.any(w["bk"]))
    use_bv = bool(np.any(w["bv"]))
    nc = _get_nc(use_bq, use_bk, use_bv)

    indices = np.asarray(inputs["indices"]).astype(np.int32)
    shared = {k: w[k] for k in ("tok", "pos", "wq", "wk", "wv", "bq", "bk", "bv",
                                "w1", "b1", "w2", "b2")}
    in_maps = [dict(shared, idx=indices[c].reshape(S, 1)) for c in range(B)]
    res = run_bass_kernel_spmd(nc, in_maps, core_ids=list(range(B)), trace=False)
    return np.concatenate([res.results[c]["out"] for c in range(B)], axis=0)


if __name__ == "__main__":
    rng = np.random.default_rng(0)
    fake = {
        "indices": rng.integers(0, V, (B, S)).astype(np.int32),
        "tok_emb": (rng.standard_normal((V, E)) * 0.02).astype(np.float32),
        "pos_emb": (rng.standard_normal((V, E)) * 0.02).astype(np.float32),
        "ln0_g": np.ones(E, np.float32), "ln0_b": np.zeros(E, np.float32),
        "Wq": (rng.standard_normal((L, E, E)) * 0.02).astype(np.float32),
        "bq": np.zeros((L, E), np.float32),
        "Wk": (rng.standard_normal((L, E, E)) * 0.02).astype(np.float32),
        "bk": np.zeros((L, E), np.float32),
        "Wv": (rng.standard_normal((L, E, E)) * 0.02).astype(np.float32),
        "bv": np.zeros((L, E), np.float32),
        "lng": np.ones((L, E), np.float32), "lnb": np.zeros((L, E), np.float32),
        "W1": (rng.standard_normal((E, HID)) * 0.02).astype(np.float32),
        "b1": np.zeros(HID, np.float32),
        "W2": (rng.standard_normal((HID, OUT)) * 0.02).astype(np.float32),
        "b2": np.zeros(OUT, np.float32),
    }
    out = kernel(**fake)
    print(out)


# revision 8
# speedup vs baseline: 55.8472x; 55.8472x over previous
"""Trainium2 Bass kernel for a 6-layer dense transformer encoder.

Sharding: data-parallel over batch — B=8 sequences, one per NeuronCore.
Each core runs the full model on its [1024] token slice; weights are
replicated. No collectives; host gathers the per-core [1,5] outputs.

Device dataflow per core (all matmuls bf16 with fp32 PSUM accumulation):
  - embedding gather (indirect DMA) + pos add + LN0
  - per layer: PE-transpose z -> xT; QT/KT/V projections; per head
    scoresT = K @ Q^T (k on partitions), exp on ScalarE, ctx via
    lhsT=expT stationary with a ones-column appended to V to produce
    softmax denominators for free; normalize; LN.
  - mean-pool (free-dim reduce on xT) -> MLP -> [1, 5]

LayerNorm gain/bias and the 1/sqrt(d) score scale and 1/S pooling are
folded into the weights on the host (exact, fp64), so the device LN is
just (x - mean) * rsqrt(var + eps).
"""

import numpy as np
import ml_dtypes

import concourse.bass as bass
import concourse.tile as tile
import concourse.mybir as mybir
from concourse import bacc
from concourse.bass_utils import run_bass_kernel_spmd
from concourse.masks import make_identity

V, E, H, L = 32000, 768, 12, 6
HID, OUT = 3072, 5
B, S = 8, 1024
D = 64
EPS = 1e-5
P = 128
KO = E // P    # 6 contraction tiles over the model dim
ST = S // P    # 8 sequence tiles of 128
NQ = S // 512  # 2 free-dim halves of the sequence
NH = HID // 512
KH = HID // P  # 24

f32 = mybir.dt.float32
bf16 = mybir.dt.bfloat16
i32 = mybir.dt.int32
AF = mybir.ActivationFunctionType
ALU = mybir.AluOpType

_NC_CACHE = {}


def _layernorm(nc, sb_small, in_ap, out_ap, eps_tile):
    """out = (in - mean(in)) * rsqrt(var(in) + eps) along 768-wide free dim."""
    stats = sb_small.tile([P, 3, 6], f32, tag="ln_stats")
    xv = in_ap.rearrange("p (c d) -> p c d", c=3)
    for c in range(3):
        nc.vector.bn_stats(out=stats[:, c, :], in_=xv[:, c, :])
    mv = sb_small.tile([P, 2], f32, tag="ln_mv")
    nc.vector.bn_aggr(out=mv[:], in_=stats[:])
    rstd = sb_small.tile([P, 1], f32, tag="ln_rstd")
    nc.scalar.activation(out=rstd[:], in_=mv[:, 1:2], func=AF.Sqrt,
                         bias=eps_tile[:], scale=1.0)
    nc.vector.reciprocal(out=rstd[:], in_=rstd[:])
    nc.vector.tensor_scalar(out_ap, in_ap, mv[:, 0:1], rstd[:],
                            ALU.subtract, ALU.mult)


def build_nc(use_bq, use_bk, use_bv, n_layers=L, with_head=True, with_attn=True, n_iters=1):
    nc = bacc.Bacc("TRN2", target_bir_lowering=False, debug=False)

    idx_d = nc.dram_tensor("idx", [S, 1], i32, kind="ExternalInput")
    tok_d = nc.dram_tensor("tok", [V, E], f32, kind="ExternalInput")
    pos_d = nc.dram_tensor("pos", [S, E], f32, kind="ExternalInput")
    wq_d = nc.dram_tensor("wq", [L, E, E], bf16, kind="ExternalInput")
    wk_d = nc.dram_tensor("wk", [L, E, E], bf16, kind="ExternalInput")
    wv_d = nc.dram_tensor("wv", [L, E, E], bf16, kind="ExternalInput")
    bq_d = nc.dram_tensor("bq", [L, E], f32, kind="ExternalInput")
    bk_d = nc.dram_tensor("bk", [L, E], f32, kind="ExternalInput")
    bv_d = nc.dram_tensor("bv", [L, E], f32, kind="ExternalInput")
    w1_d = nc.dram_tensor("w1", [E, HID], bf16, kind="ExternalInput")
    b1_d = nc.dram_tensor("b1", [1, HID], f32, kind="ExternalInput")
    w2_d = nc.dram_tensor("w2", [HID, OUT], bf16, kind="ExternalInput")
    b2_d = nc.dram_tensor("b2", [1, OUT], f32, kind="ExternalInput")
    out_d = nc.dram_tensor("out", [1, OUT], f32, kind="ExternalOutput")

    from contextlib import ExitStack
    with tile.TileContext(nc) as tc:
        with ExitStack() as ctx:
            ent = ctx.enter_context
            consts = ent(tc.tile_pool(name="consts", bufs=1))
            sb_small = ent(tc.tile_pool(name="sb_small", bufs=4))
            embp = ent(tc.tile_pool(name="embp", bufs=2))
            zp = ent(tc.tile_pool(name="zp", bufs=2))
            xnewp = ent(tc.tile_pool(name="xnewp", bufs=1))
            xtp = ent(tc.tile_pool(name="xtp", bufs=1))
            qtp = ent(tc.tile_pool(name="qtp", bufs=1))
            ktp = ent(tc.tile_pool(name="ktp", bufs=1))
            vp = ent(tc.tile_pool(name="vp", bufs=2))
            expp = ent(tc.tile_pool(name="expp", bufs=2))
            wp = ent(tc.tile_pool(name="wp", bufs=1))
            headp = ent(tc.tile_pool(name="headp", bufs=1))
            ps_proj = ent(tc.tile_pool(name="ps_proj", bufs=2, space="PSUM"))
            ps_sc = ent(tc.tile_pool(name="ps_sc", bufs=2, space="PSUM"))
            ps_sm = ent(tc.tile_pool(name="ps_sm", bufs=2, space="PSUM"))

            def emit_body():
                _emit(nc, tc, consts, sb_small, embp, zp, xnewp, xtp, qtp, ktp,
                      vp, expp, wp, headp, ps_proj, ps_sc, ps_sm,
                      idx_d, tok_d, pos_d, wq_d, wk_d, wv_d, bq_d, bk_d, bv_d,
                      w1_d, b1_d, w2_d, b2_d, out_d,
                      use_bq, use_bk, use_bv, n_layers, with_head)
            if n_iters == 1:
                emit_body()
            else:
                with tc.For_i(0, n_iters, 1):
                    emit_body()

    nc.compile()
    return nc


def _emit(nc, tc, consts, sb_small, embp, zp, xnewp, xtp, qtp, ktp,
          vp, expp, wp, headp, ps_proj, ps_sc, ps_sm,
          idx_d, tok_d, pos_d, wq_d, wk_d, wv_d, bq_d, bk_d, bv_d,
          w1_d, b1_d, w2_d, b2_d, out_d,
          use_bq, use_bk, use_bv, n_layers, with_head):
    if True:
        if True:
            ident = consts.tile([P, P], bf16)
            make_identity(nc, ident[:])
            eps_t = consts.tile([P, 1], f32)
            nc.vector.memset(eps_t[:], EPS)
            idx_sb = consts.tile([P, ST], i32)
            nc.sync.dma_start(idx_sb[:], idx_d.ap().rearrange("(t p) o -> p (t o)", p=P))

            # ---- embedding + LN0 -> z ----
            z = zp.tile([P, ST, E], bf16, tag="z")
            for st in range(ST):
                emb = embp.tile([P, E], f32, tag="emb")
                nc.gpsimd.indirect_dma_start(
                    out=emb[:], out_offset=None, in_=tok_d.ap(),
                    in_offset=bass.IndirectOffsetOnAxis(ap=idx_sb[:, st:st + 1], axis=0),
                )
                pos = embp.tile([P, E], f32, tag="pos")
                nc.sync.dma_start(pos[:], pos_d.ap()[st * P:(st + 1) * P, :])
                nc.vector.tensor_add(out=emb[:], in0=emb[:], in1=pos[:])
                _layernorm(nc, sb_small, emb[:], z[:, st, :], eps_t)

            # ---- transformer layers ----
            for l in range(n_layers):
                wq_sb = wp.tile([P, KO, E], bf16, tag="wq")
                nc.sync.dma_start(wq_sb[:], wq_d.ap()[l].rearrange("(ko p) f -> p ko f", p=P))
                wk_sb = wp.tile([P, KO, E], bf16, tag="wk")
                nc.sync.dma_start(wk_sb[:], wk_d.ap()[l].rearrange("(ko p) f -> p ko f", p=P))
                wv_sb = wp.tile([P, KO, E], bf16, tag="wv")
                nc.sync.dma_start(wv_sb[:], wv_d.ap()[l].rearrange("(ko p) f -> p ko f", p=P))
                if use_bq:
                    bq_sb = sb_small.tile([P, KO], f32, tag="bq")
                    nc.sync.dma_start(bq_sb[:], bq_d.ap()[l].rearrange("(ko p) -> p ko", p=P))
                if use_bk:
                    bk_sb = sb_small.tile([P, KO], f32, tag="bk")
                    nc.sync.dma_start(bk_sb[:], bk_d.ap()[l].rearrange("(ko p) -> p ko", p=P))
                if use_bv:
                    bv_bc = sb_small.tile([P, E], f32, tag="bv")
                    src = bv_d.ap()[l]
                    nc.sync.dma_start(bv_bc[:], bass.AP(
                        tensor=src.tensor, offset=src.offset, ap=[[0, P], *src.ap]))

                # z -> xT (PE transpose per 128x128 block)
                xT = xtp.tile([P, KO, S], bf16, tag="xT")
                for ko in range(KO):
                    for st in range(ST):
                        tp = ps_sm.tile([P, P], bf16, tag="sm")
                        nc.tensor.transpose(tp[:], z[:, st, ko * P:(ko + 1) * P], ident[:])
                        nc.vector.tensor_copy(xT[:, ko, st * P:(st + 1) * P], tp[:])

                # QT/KT projections: [e_out partitions, s free]
                QT = qtp.tile([P, KO, S], bf16, tag="QT")
                KT = ktp.tile([P, KO, S], bf16, tag="KT")
                for dst, w_sb, b_sb in ((QT, wq_sb, bq_sb if use_bq else None),
                                        (KT, wk_sb, bk_sb if use_bk else None)):
                    for eo in range(KO):
                        for qh in range(NQ):
                            pq = ps_proj.tile([P, 512], f32, tag="proj")
                            for ko in range(KO):
                                nc.tensor.matmul(
                                    pq[:], w_sb[:, ko, eo * P:(eo + 1) * P],
                                    xT[:, ko, qh * 512:(qh + 1) * 512],
                                    start=(ko == 0), stop=(ko == KO - 1))
                            o = dst[:, eo, qh * 512:(qh + 1) * 512]
                            if b_sb is not None:
                                nc.vector.tensor_scalar_add(o, pq[:], b_sb[:, eo:eo + 1])
                            else:
                                nc.vector.tensor_copy(o, pq[:])

                # V projection in natural layout with ones column: [s part, kt, h, 65]
                Vp = vp.tile([P, ST, H, D + 1], bf16, tag="Vp")
                nc.vector.memset(Vp[:, :, :, D:D + 1], 1.0)
                for st in range(ST):
                    for half in range(2):
                        pv = ps_proj.tile([P, 384], f32, tag="proj")
                        for ko in range(KO):
                            nc.tensor.matmul(
                                pv[:], xT[:, ko, st * P:(st + 1) * P],
                                wv_sb[:, ko, half * 384:(half + 1) * 384],
                                start=(ko == 0), stop=(ko == KO - 1))
                        o = Vp[:, st, half * 6:(half + 1) * 6, 0:D]
                        pvv = pv[:].rearrange("p (h d) -> p h d", h=6)
                        if use_bv:
                            bvv = bv_bc[:, half * 384:(half + 1) * 384].rearrange(
                                "p (h d) -> p h d", h=6)
                            nc.vector.tensor_tensor(out=o, in0=pvv, in1=bvv, op=ALU.add)
                        else:
                            nc.vector.tensor_copy(o, pvv)

                # attention + ctx + normalize
                xnew = xnewp.tile([P, ST, E], bf16, tag="xnew")
                for h in range(H):
                    po = (h % 2) * D
                    eo = h // 2
                    expT = expp.tile([P, ST, S], bf16, tag="expT")
                    for kt in range(ST):
                        sc = ps_sc.tile([P, 1024], f32, tag="sc")
                        for qh in range(NQ):
                            nc.tensor.matmul(
                                sc[:, qh * 512:(qh + 1) * 512],
                                KT[po:po + D, eo, kt * P:(kt + 1) * P],
                                QT[po:po + D, eo, qh * 512:(qh + 1) * 512])
                        nc.scalar.activation(out=expT[:, kt, :], in_=sc[:],
                                             func=AF.Exp, bias=0.0, scale=1.0)
                    for qt in range(ST):
                        ct = ps_sm.tile([P, D + 1], f32, tag="sm")
                        for kt in range(ST):
                            nc.tensor.matmul(
                                ct[:], expT[:, kt, qt * P:(qt + 1) * P],
                                Vp[:, kt, h, :],
                                start=(kt == 0), stop=(kt == ST - 1))
                        rec = sb_small.tile([P, 1], f32, tag="rec")
                        nc.vector.reciprocal(rec[:], ct[:, D:D + 1])
                        nc.vector.tensor_scalar_mul(
                            xnew[:, qt, h * D:(h + 1) * D], ct[:, 0:D], rec[:])

                # LN -> z for next layer
                z = zp.tile([P, ST, E], bf16, tag="z")
                for st in range(ST):
                    _layernorm(nc, sb_small, xnew[:, st, :], z[:, st, :], eps_t)

            # ---- head: mean-pool (sum; 1/1024 folded in w1) + MLP ----
            if not with_head:
                o_sb = headp.tile([1, OUT], f32, tag='o_sb')
                nc.vector.memset(o_sb[:], 0.0)
                nc.vector.tensor_scalar_add(o_sb[0, 0:1], z[0, 0, 0:1], 0.0)
                nc.sync.dma_start(out_d.ap(), o_sb[:])
            xTf = xtp.tile([P, KO, S], bf16, tag="xT")
            for ko in range(KO):
                for st in range(ST):
                    tp = ps_sm.tile([P, P], bf16, tag="sm")
                    nc.tensor.transpose(tp[:], z[:, st, ko * P:(ko + 1) * P], ident[:])
                    nc.vector.tensor_copy(xTf[:, ko, st * P:(st + 1) * P], tp[:])
            pooled_f = headp.tile([P, KO], f32, tag="pooled_f")
            nc.vector.reduce_sum(out=pooled_f[:], in_=xTf[:], axis=mybir.AxisListType.X)
            pooled = headp.tile([P, KO], bf16, tag="pooled")
            nc.vector.tensor_copy(pooled[:], pooled_f[:])

            hT_pre = headp.tile([P, KH], f32, tag="hT_pre")
            for nt in range(NH):
                w1_sb = wp.tile([P, KO, 512], bf16, tag="wq")
                nc.sync.dma_start(
                    w1_sb[:], w1_d.ap().rearrange("(ko p) f -> p ko f", p=P)[:, :, nt * 512:(nt + 1) * 512])
                # hT block = W1_tile.T @ pooledT  -> lands already transposed
                for hsub in range(4):
                    phT = ps_sm.tile([P, 1], f32, tag="sm")
                    for ko in range(KO):
                        nc.tensor.matmul(phT[:], w1_sb[:, ko, hsub * P:(hsub + 1) * P],
                                         pooled[:, ko:ko + 1],
                                         start=(ko == 0), stop=(ko == KO - 1))
                    nc.vector.tensor_copy(hT_pre[:, nt * 4 + hsub:nt * 4 + hsub + 1], phT[:])
            b1T = headp.tile([P, KH], f32, tag="b1T")
            nc.sync.dma_start(b1T[:], b1_d.ap()[0].rearrange("(ko p) -> p ko", p=P))
            nc.vector.tensor_add(out=hT_pre[:], in0=hT_pre[:], in1=b1T[:])
            hT = headp.tile([P, KH], bf16, tag="hT")
            nc.vector.tensor_scalar_max(hT[:], hT_pre[:], 0.0)

            w2_sb = headp.tile([P, KH, OUT], bf16, tag="w2_sb")
            nc.sync.dma_start(w2_sb[:], w2_d.ap().rearrange("(ko p) f -> p ko f", p=P))
            b2_sb = headp.tile([1, OUT], f32, tag="b2_sb")
            nc.sync.dma_start(b2_sb[:], b2_d.ap())
            po2 = ps_sm.tile([1, OUT], f32, tag="sm")
            for ko in range(KH):
                nc.tensor.matmul(po2[:], hT[:, ko:ko + 1], w2_sb[:, ko, :],
                                 start=(ko == 0), stop=(ko == KH - 1))
            o_sb = headp.tile([1, OUT], f32, tag="o_sb")
            nc.vector.tensor_add(out=o_sb[:], in0=po2[:], in1=b2_sb[:])
            nc.sync.dma_start(out_d.ap(), o_sb[:])


def _get_nc(use_bq, use_bk, use_bv):
    key = (use_bq, use_bk, use_bv)
    if key not in _NC_CACHE:
        _NC_CACHE[key] = build_nc(*key)
    return _NC_CACHE[key]


def prep_weights(inputs):
    """Fold LN affine params, score scale and pooling mean into the weights."""
    f8 = np.float64
    Wq = np.asarray(inputs["Wq"], f8)
    Wk = np.asarray(inputs["Wk"], f8)
    Wv = np.asarray(inputs["Wv"], f8)
    bq = np.asarray(inputs["bq"], f8)
    bk = np.asarray(inputs["bk"], f8)
    bv = np.asarray(inputs["bv"], f8)
    lng = np.asarray(inputs["lng"], f8)
    lnb = np.asarray(inputs["lnb"], f8)
    g_prev = np.concatenate([np.asarray(inputs["ln0_g"], f8)[None], lng[:L - 1]], 0)
    b_prev = np.concatenate([np.asarray(inputs["ln0_b"], f8)[None], lnb[:L - 1]], 0)

    scale = 1.0 / np.sqrt(D)
    wq_eff = g_prev[:, :, None] * Wq * scale
    bq_eff = (bq + np.einsum("le,lef->lf", b_prev, Wq)) * scale
    wk_eff = g_prev[:, :, None] * Wk
    bk_eff = bk + np.einsum("le,lef->lf", b_prev, Wk)
    wv_eff = g_prev[:, :, None] * Wv
    bv_eff = bv + np.einsum("le,lef->lf", b_prev, Wv)

    W1 = np.asarray(inputs["W1"], f8)
    w1_eff = lng[L - 1][:, None] * W1 / S
    b1_eff = np.asarray(inputs["b1"], f8) + lnb[L - 1] @ W1

    bf = ml_dtypes.bfloat16
    return {
        "wq": wq_eff.astype(bf), "wk": wk_eff.astype(bf), "wv": wv_eff.astype(bf),
        "bq": bq_eff.astype(np.float32), "bk": bk_eff.astype(np.float32),
        "bv": bv_eff.astype(np.float32),
        "w1": w1_eff.astype(bf), "b1": b1_eff.astype(np.float32)[None, :],
        "w2": np.asarray(inputs["W2"], f8).astype(bf),
        "b2": np.asarray(inputs["b2"], f8).astype(np.float32)[None, :],
        "tok": np.asarray(inputs["tok_emb"], np.float32),
        "pos": np.asarray(inputs["pos_emb"], np.float32)[:S],
    }


def kernel(**inputs) -> np.ndarray:
    w = prep_weights(inputs)
    use_bq = bool(np.any(w["bq"]))
    use_bk = bool(np.any(w["bk"]))
    use_bv = bool(np.any(w["bv"]))
    nc = _get_nc(use_bq, use_bk, use_bv)

    indices = np.asarray(inputs["indices"]).astype(np.int32)
    shared = {k: w[k] for k in ("tok", "pos", "wq", "wk", "wv", "bq", "bk", "bv",
                                "w1", "b1", "w2", "b2")}
    in_maps = [dict(shared, idx=indices[c].reshape(S, 1)) for c in range(B)]
    res = run_bass_kernel_spmd(nc, in_maps, core_ids=list(range(B)), trace=False)
    return np.concatenate([res.results[c]["out"] for c in range(B)], axis=0)


if __name__ == "__main__":
    rng = np.random.default_rng(0)
    fake = {
        "indices": rng.integers(0, V, (B, S)).astype(np.int32),
        "tok_emb": (rng.standard_normal((V, E)) * 0.02).astype(np.float32),
        "pos_emb": (rng.standard_normal((V, E)) * 0.02).astype(np.float32),
        "ln0_g": np.ones(E, np.float32), "ln0_b": np.zeros(E, np.float32),
        "Wq": (rng.standard_normal((L, E, E)) * 0.02).astype(np.float32),
        "bq": np.zeros((L, E), np.float32),
        "Wk": (rng.standard_normal((L, E, E)) * 0.02).astype(np.float32),
        "bk": np.zeros((L, E), np.float32),
        "Wv": (rng.standard_normal((L, E, E)) * 0.02).astype(np.float32),
        "bv": np.zeros((L, E), np.float32),
        "lng": np.ones((L, E), np.float32), "lnb": np.zeros((L, E), np.float32),
        "W1": (rng.standard_normal((E, HID)) * 0.02).astype(np.float32),
        "b1": np.zeros(HID, np.float32),
        "W2": (rng.standard_normal((HID, OUT)) * 0.02).astype(np.float32),
        "b2": np.zeros(OUT, np.float32),
    }
    out = kernel(**fake)
    print(out)
